# revision 84
# baseline (speedup 1.0000x reference)
"""Distributed GQA attention (llama-style RoPE) for one TRN2 chip (8 NeuronCores).

Sharding: core c handles batch b=c//4 and sequence-quarter q=c%4 (512 q-rows).
Each core projects Q for its own rows (all 32 heads), projects K/V for its own
512 positions, AllGathers K/V within its 4-core batch group, runs attention for
its rows, and applies the output projection. Output rows are disjoint across
cores, so no all-reduce is needed; the host concatenates.

On-chip dataflow (per core):
  xT   = x.T (pre-transposed on host, landed in dc chunks)        [d, rows]
  kT   = wk.T @ xT -> RoPE -> fp16 -> AllGather (per feature chunk,
         dispatched as soon as that chunk's rope is done) -> [feat, skv]
  v    = xT.T @ wv -> bf16, shipped 520 wide (a ones column after each
         head's 64 cols so the AV matmul accumulates the softmax denom
         in the same instruction) -> AllGather -> [skv, 8*65]
  per head pair:
    qT   = wq.T @ xT -> RoPE -> fp16                 [feat, sq]
    sH   = kT_h.T @ qT_h  (per-head 1-bank psum tiles so the exp of head
           A frees its bank while head B's exp still runs)  [skv, sq]
    e    = exp(sH/8): ACT activation -> bf16 on most chunks; on
           dve_lanes(pair) chunks DVE computes the Schraudolph bit-trick
           bits = s*23.083 + 16250 -> int16, bitcast bf16 (~3% sawtooth,
           measured rel-err 1.15e-2 vs 2e-2 gate) - this splits the
           265us exp stream across both engines
    oTr  = e_blk.T @ [v|1]  accum over skv chunks    [sq, 65]  psum f32
    o_n  = oTr[:, :64] * recip(oTr[:, 64]) -> bf16: recips on DVE straight
           from psum, psum->sbuf copies split ACT/DVE, muls split DVE/Pool
    oT   = PE-transpose(o_n)                         [feat, sq]
  out  = oT.T @ wo, 4 head-group partials accumulated in SBUF via DVE
         (GPSIMD and DMA cannot access PSUM on TRN2), streamed to DRAM.

Schedule: every engine queue executes IN ORDER, so instruction emission order
is the schedule.  Per pair (18-step chunk loop, attn.V lagging 2 chunks):
  c0-c7 : next q-projection, 2 dc-chunks per step (levels PE's per-chunk
          load against the ~1.04us exp cadence); pairs 0-3 carry a second
          q-projection at c8-15 on pslot(1)
  c0/c5/c7: previous pair's normalize / PE-transposes / copy-to-oT - the
          transposes sit IN the PE queue, so they are placed after the
          ~4.5us cross-engine normalize chain can complete
  c7,9,11,13: out-proj quarters of the previous group (c3: wo prefetch;
          c10: wq prefetch; pair 15 also prefetches group 3's wo tiles)
PSUM (8 banks): per-head scores 4 bufs x 1 bank; pslots ppA/ppB (q-proj,
transposes, out-proj tiles - explicit tags); AV accum 2 x 1 bank holding 4
[128,65] f32 regions each (start=True only on region 0 per bank).

All weights are pre-swizzled on the HOST into the exact SBUF layouts so every
DMA is a fully-linear copy; the first-needed bytes are emitted first since
the DMA bus serves transfers in arrival order. RoPE uses the half-rotated
layout (head features [evens|odds]); cos/sin ship as fp16 CC/SS [128, sq]
with SS block-pair-swapped because SB-SB DVE ops need equal input base
partitions (only outputs may partition-shift). In solo (sim) mode the
AllGathers are stand-in broadcast DMAs moving the same byte volume.
"""
import sys

sys.path.insert(0, "/opt/trn_rl_repo")

import numpy as np
import ml_dtypes
from contextlib import ExitStack

import concourse.bass as bass
import concourse.mybir as mybir
import concourse.tile as tile
from concourse import bacc
from concourse.bass_utils import run_bass_kernel_spmd
from concourse.masks import make_identity

B, S, D = 2, 2048, 2048
NQ, NKV, HD = 32, 8, 64
NCORES = 8
GPB = 4                 # cores per batch group
SQ = S // GPB           # 512 q-rows per core
P = 128
DC = D // P             # 16 contraction chunks
KF = NKV * HD           # 512 kv feature dim
KFC = KF // P           # 4 kv feature chunks
SC = S // P             # 16 skv chunks
RQ = SQ // P            # 4 q-row blocks
AW = HD + 1             # AV width: 64 v cols + 1 ones col (softmax denom)
VW = NKV * AW           # V ships 520 wide: ones column after each head

FP = mybir.dt.float32
BF = mybir.dt.bfloat16
F16 = mybir.dt.float16
I16 = mybir.dt.int16
EXPF = mybir.ActivationFunctionType.Exp
EXP_SCALE = 1.0 / 8.0   # 1/sqrt(HD)

# DVE-lane "exp": Schraudolph bit-trick in bf16 space. bits = s*SL + SB,
# written as int16 and bitcast to bf16: 2^((bits-16256)/128) ~= exp(s/8).
# DVE convert rounds-to-nearest (verified on device); ~3% sawtooth error,
# applied to dve_lanes(pair) of the 16 kv-chunks per pair (rel-err impact
# measured 1.0e-2 at 4/16 uniform vs 4.9e-3 all-ACT baseline; gate 2e-2).
SL = 128.0 * 1.44269504 / 8.0     # 23.083 bits per raw-score unit
SB = 16250.0
MUL = mybir.AluOpType.mult
ADDOP = mybir.AluOpType.add


def dve_lanes(pair):
    """Chunks whose exp runs on DVE instead of ACT. Late pairs have less PE
    work per chunk (no qproj filler), so ACT needs more relief there."""
    if pair <= 1:
        return (3, 6, 9, 12, 15)
    if pair <= 3:
        return (3, 7, 11, 14)
    if pair <= 9:
        return (2, 6, 10, 14)
    return (2, 4, 8, 10, 12, 14)


def build(solo=False):
    nc = bacc.Bacc("TRN2", target_bir_lowering=False, debug=False,
                   num_devices=1 if solo else NCORES)

    x_e = nc.dram_tensor("x", [P, DC, SQ], BF, kind="ExternalInput").ap()
    wq_e = nc.dram_tensor("wq", [DC, P, DC, P], BF, kind="ExternalInput").ap()
    wk_e = nc.dram_tensor("wk", [KFC, P, DC, P], BF, kind="ExternalInput").ap()
    wv_e = nc.dram_tensor("wv", [P, DC, KF], BF, kind="ExternalInput").ap()
    wo_e = nc.dram_tensor("wo", [RQ, RQ, P, RQ, 512], BF, kind="ExternalInput").ap()
    cc_e = nc.dram_tensor("cc", [P, SQ], F16, kind="ExternalInput").ap()
    ss_e = nc.dram_tensor("ss", [P, SQ], F16, kind="ExternalInput").ap()
    out_e = nc.dram_tensor("out", [SQ, D], FP, kind="ExternalOutput").ap()

    groups = [[0, 1, 2, 3], [4, 5, 6, 7]]

    with tile.TileContext(nc) as tc, ExitStack() as ctx:
        sb = ctx.enter_context(tc.tile_pool(name="sb", bufs=1))
        rp = ctx.enter_context(tc.tile_pool(name="rp", bufs=3))
        epool = ctx.enter_context(tc.tile_pool(name="epool", bufs=12))
        npool = ctx.enter_context(tc.tile_pool(name="npool", bufs=3))
        opool = ctx.enter_context(tc.tile_pool(name="opool", bufs=5))
        otp = ctx.enter_context(tc.tile_pool(name="otp", bufs=2))
        early = ctx.enter_context(tc.tile_pool(name="early", bufs=1))
        wqp = ctx.enter_context(tc.tile_pool(name="wqp", bufs=6))
        dram = ctx.enter_context(tc.tile_pool(name="dram", bufs=1, space="DRAM"))
        pp = ctx.enter_context(tc.tile_pool(name="pp", bufs=1, space="PSUM"))
        psc = ctx.enter_context(tc.tile_pool(name="psc", bufs=4, space="PSUM"))
        av = ctx.enter_context(tc.tile_pool(name="av", bufs=2, space="PSUM"))

        # ---- constants ----
        cc_sb = sb.tile([P, SQ], F16)
        ss_sb = sb.tile([P, SQ], F16)
        id_sb = sb.tile([P, P], BF)

        def rope_chunk(ps, dst, act=True):
            """dst = RoPE(ps) in half-rotated layout; ps [128,SQ] psum f32.
            ACT (or DVE when ACT is the loaded engine) stages psum->fp16 sbuf
            freeing the psum slot early; all-fp16 DVE ops run in 2x mode."""
            c16 = rp.tile([P, SQ], F16, tag="c16")
            if act:
                nc.scalar.copy(c16[:], ps[:])
            else:
                nc.vector.tensor_copy(c16[:], ps[:])
            t0 = rp.tile([P, SQ], F16, tag="t0")
            t1 = rp.tile([P, SQ], F16, tag="t1")
            nc.vector.tensor_mul(t0[:], c16[:], cc_sb[:])
            # SB-SB ops need equal input base partitions: SS is block-swapped
            # on the host so both inputs read at i0 and only the OUT shifts
            for blk in range(4):
                o0, i0 = blk * 32, (blk ^ 1) * 32
                nc.vector.tensor_mul(t1[o0:o0 + 32, :], c16[i0:i0 + 32, :],
                                     ss_sb[i0:i0 + 32, :])
            nc.vector.tensor_add(dst, t0[:], t1[:])

        qT = sb.tile([P, DC, SQ], F16)
        kag_in = dram.tile([KFC, P, SQ], F16)
        kag_out = dram.tile([KFC, GPB, P, SQ], F16)
        vag_in = dram.tile([RQ, P, VW], BF)
        vag_out = dram.tile([RQ, GPB, P, VW], BF)

        # ---- input loads: the DMA bus is one shared 360GB/s resource in
        #      arrival order, so emit the bytes the pipeline needs first:
        #      x/wk in interleaved quarter-chunks (kproj contracts as they
        #      land), then rope tables, then wq0, then wv ----
        xT = early.tile([P, DC, SQ], BF, tag="xT", name="xT")
        wkq = [early.tile([P, DC, P], BF, tag="wkqf", bufs=KFC,
                          name=f"wkq{fc}") for fc in range(KFC)]
        # Three parallel DMA streams: x on SP (+Pool for the late quarters),
        # wk/rope-tables/wq1 on ACT, wq0/wv on SP after x. First dc-chunks
        # land alone so kproj's first matmuls start early.
        nc.sync.dma_start(xT[:, 0:2, :], x_e[:, 0:2, :])
        nc.scalar.dma_start(wkq[0][:, 0:4, :], wk_e[0][:, 0:4, :])
        nc.gpsimd.dma_start(xT[:, 4:8, :], x_e[:, 4:8, :])
        nc.sync.dma_start(xT[:, 2:4, :], x_e[:, 2:4, :])
        nc.scalar.dma_start(wkq[0][:, 4:16, :], wk_e[0][:, 4:16, :])
        nc.sync.dma_start(xT[:, 8:12, :], x_e[:, 8:12, :])
        nc.gpsimd.dma_start(xT[:, 12:16, :], x_e[:, 12:16, :])
        nc.scalar.dma_start(wkq[1][:], wk_e[1])
        nc.scalar.dma_start(cc_sb[:], cc_e)
        nc.scalar.dma_start(ss_sb[:], ss_e)
        wq_tiles = {}

        def wq_prefetch(pair, q=None):
            w = wqp.tile([P, DC, P], BF, tag="wq", name=f"wq_{pair}")
            (q or nc.sync).dma_start(w[:], wq_e[pair])
            wq_tiles[pair] = w

        nc.scalar.dma_start(wkq[2][:], wk_e[2])
        nc.scalar.dma_start(wkq[3][:], wk_e[3])
        wv_sb = early.tile([P, DC, KF], BF, tag="big32", name="wv_sb")
        for wvq in range(4):   # quarters: vproj matmuls start on the first
            nc.sync.dma_start(wv_sb[:, wvq * 4:(wvq + 1) * 4, :],
                              wv_e[:, wvq * 4:(wvq + 1) * 4, :])
        wq_prefetch(0, nc.scalar)   # ACT stream has slack; sync is x+wv-bound
        wq_prefetch(1, nc.scalar)
        make_identity(nc, id_sb)
        # Warm the PE through its p-state ramp during the otherwise-idle
        # input-DMA wait: ~240ns per throwaway matmul (MID-clock 107ns +
        # same-tile WAR ack), 22 of them end just past the ~5.44us wk0a/x01
        # landing, so kproj starts gap-free at the full 2.4GHz instead of
        # 0.65-1.2GHz. Results are never read. (8 ended too early - the idle
        # gap reset the ramp; 30 overshot and queued kproj behind them.)
        # pp0 slot, NOT psc: a psc alloc would shift the score-tile buffer
        # rotation for the whole run; pp0's next user is kproj(0), which is
        # exactly what should queue right behind the warmup.
        warm = pp.tile([P, 512], FP, tag="pp0", name="warm")
        for _ in range(22):
            nc.tensor.matmul(warm[:, 0:P], lhsT=id_sb[:], rhs=id_sb[:],
                             start=True, stop=True)

        def pslot(slot, dtype=FP, shape=(P, 512), name="ps"):
            return pp.tile(list(shape), dtype, tag=f"pp{slot}", name=name)

        qproj_ps = {}

        def qproj_mm(pair, dcs, qps):
            """Emit dc-chunk matmuls of the Q projection for `pair`; rope and
            release the psum slot after the last chunk."""
            if qps is not None:
                qproj_ps[pair] = (wq_tiles.pop(pair), qps)
            wq_sb, qps = qproj_ps[pair]
            for dc in dcs:
                nc.tensor.matmul(qps[:, :SQ], lhsT=wq_sb[:, dc, :],
                                 rhs=xT[:, dc, :],
                                 start=(dc == 0), stop=(dc == DC - 1))
            if dcs[-1] == DC - 1:
                rope_chunk(qps[:, :SQ], qT[:, pair, :])
                del qproj_ps[pair]

        # ---- K projection + RoPE first; AllGather per feature chunk so the
        #      gather pipeline overlaps the remaining ropes.  qproj(0) sits
        #      between the kproj halves: its rope (which gates pair-0 scores)
        #      lands 3rd in the DVE chain, while AllGathers for the late kv
        #      chunks (first needed by pairs 2/3) may finish late ----
        kT_own = sb.tile([P, KFC, SQ], F16, tag="own4", name="kT_own")

        def kproj(fc):
            ps = pslot(fc % 2, name="kps")
            for dc in range(DC):
                nc.tensor.matmul(ps[:, :SQ],
                                 lhsT=wkq[fc][:, dc, :],
                                 rhs=xT[:, dc, :],
                                 start=(dc == 0), stop=(dc == DC - 1))
            rope_chunk(ps[:, :SQ], kT_own[:, fc, :])
            # fc0 feeds pair 0: its staging + landing ride the ACT queue
            # (ahead of the exp stream, behind nothing slow); fc1 rides Pool
            # (idle until the V gathers) so the weight stream on SP is never
            # blocked behind a gather wait; fc2/3 go via SP late
            q = (nc.scalar, nc.gpsimd, nc.sync, nc.sync)[fc]
            q.dma_start(kag_in[fc], kT_own[:, fc, :])
            if solo:
                # stand-in for the AllGather: one broadcast DMA moves the
                # same byte volume without 4 serialized trigger overheads
                q.dma_start(kag_out[fc],
                            kag_in[fc:fc + 1].broadcast_to((GPB, P, SQ)))
            else:
                nc.gpsimd.collective_compute(
                    "AllGather", mybir.AluOpType.bypass,
                    replica_groups=groups,
                    ins=[kag_in[fc]], outs=[kag_out[fc]])
            q.dma_start(kT[:, fc, :],
                        kag_out[fc].rearrange("r p s -> p r s"))

        kT = early.tile([P, KFC, S], F16, tag="kT", name="kT")
        kproj(0)
        kproj(1)
        # pairs 2-5's wq tiles: emitted AFTER kproj(1)'s stage/land DMAs so
        # their bus transfers never jump ahead of the K path (needed ~15us
        # later than kT chunk 0)
        wq_prefetch(2)
        wq_prefetch(3)

        # ---- V projection -> AllGather, one per 128-row block so the first
        #      kv chunks reach the attention loop as early as possible.
        #      V ships 520 wide: each kv head's 64 columns are followed by a
        #      ones column, so the AV matmul accumulates the softmax denom in
        #      the same instruction (no twin N=1 matmul).
        #      PSUM: the av-pool banks (idle until pair 0's AV) - the pp
        #      slots are WAR-held by the k-ropes; copies on ACT (DVE busy) ----
        v_own = sb.tile([P, RQ, VW], BF, tag="vown", name="v_own")
        v_pos = early.tile([P, SC, VW], BF, tag="big32", name="v_pos")
        nc.gpsimd.memset(v_own[:, :, HD:VW:AW], 1.0)

        def vproj(pc):
            ps = av.tile([P, 512], FP, tag="av", name="vps")
            for dc in range(DC):
                nc.tensor.matmul(ps[:, :KF],
                                 lhsT=xT[:, dc, pc * P:(pc + 1) * P],
                                 rhs=wv_sb[:, dc, :],
                                 start=(dc == 0), stop=(dc == DC - 1))
            # first two copies on ACT (DVE busy with prologue ropes); the
            # later two land during pair 0 whose ACT queue is exp-saturated
            ceng = nc.scalar if pc < 2 else nc.vector
            (ceng.copy if pc < 2 else ceng.tensor_copy)(
                v_own[:, pc, :].rearrange("p (h w) -> p h w", h=NKV)[:, :, 0:HD],
                ps[:, :KF].rearrange("p (h f) -> p h f", h=NKV))
            # whole V path rides the Pool queue (collectives live there too)
            nc.gpsimd.dma_start(vag_in[pc], v_own[:, pc, :])
            if solo:
                nc.gpsimd.dma_start(vag_out[pc],
                                    vag_in[pc:pc + 1].broadcast_to((GPB, P, VW)))
            else:
                nc.gpsimd.collective_compute(
                    "AllGather", mybir.AluOpType.bypass,
                    replica_groups=groups,
                    ins=[vag_in[pc]], outs=[vag_out[pc]])

        # V chunk 0 right after the kprojs (wv quarters land before wq0 on
        # SP): its gather gates pair-0 AV, and the two q-projections fill
        # the PE while it runs
        vproj(0)
        qproj_mm(0, list(range(DC)),
                 psc.tile([P, SQ], FP, tag="psc", name="q0ps"))
        qproj_mm(1, list(range(DC)),
                 psc.tile([P, SQ], FP, tag="psc", name="q1ps"))
        for pc in range(1, RQ):
            vproj(pc)
        for pc in range(RQ):
            # land v chunks {pc, 4+pc, 8+pc, 12+pc} (position-major layout:
            # contiguous rows, full DMA rate); emitted after ALL the gathers
            # so a land's wait never queue-blocks the next gather trigger
            nc.gpsimd.dma_start(v_pos[:, pc:SC:GPB, :],
                                vag_out[pc].rearrange("r p f -> p r f"))

        kproj(2)
        wq_prefetch(4)
        wq_prefetch(5)
        kproj(3)

        # ---- per-pair attention loop ----
        oT_tiles = {}
        wo_g3 = {}              # group-3 wo tiles, prefetched during pair 15

        def wo_load(g, nf):
            wo_nf = opool.tile([P, 4, 512], BF, tag="wo", name="wo_nf")
            nc.sync.dma_start(wo_nf[:], wo_e[g, nf])
            return wo_nf

        out_acc = sb.tile([P, RQ, D], FP)

        def out_proj_m(g, nf, wo_nf, m, slot):
            """One [128-row, 512-col] tile of group g's out-projection,
            accumulated in SBUF; group 3 streams the finished slice out on
            alternating DMA queues."""
            oT = oT_tiles[g]
            ps = pslot(slot, name="ops")
            for ch in range(4):
                nc.tensor.matmul(ps[:],
                                 lhsT=oT[:, ch, m * P:(m + 1) * P],
                                 rhs=wo_nf[:, ch, :],
                                 start=(ch == 0), stop=(ch == 3))
            acc = out_acc[:, m, nf * 512:(nf + 1) * 512]
            if g == 0:
                nc.vector.tensor_copy(acc, ps[:])
            else:
                nc.vector.tensor_add(acc, acc, ps[:])
            if g == 3:
                # ACT/SP triggers (HWDGE ~0.6us) — Pool's SWDGE trigger costs
                # ~1us of engine time and serializes the final flush
                eng = (nc.scalar, nc.sync)[(nf + m) % 2]
                eng.dma_start(
                    out_e[m * P:(m + 1) * P, nf * 512:(nf + 1) * 512], acc)

        fin = {}                # previous pair's normalize/transpose state

        def finish_stage(stage, end=False):
            """stage 0: normalize (DVE); 1: PE-transpose; 2: copy to oT."""
            if not fin:
                return
            if stage == 0:
                # recips straight from psum (parallel to the ACT copies, not
                # after them), ACT stages the AV psum to sbuf bf16, then the 8
                # muls split DVE/Pool so the chain closes before the c==3/4
                # PE transposes.
                favA, favB = fin["avA"], fin["avB"]
                rc = npool.tile([P, 8], FP, tag="rc", name="rc")
                nc.vector.reciprocal(rc[:, 0:4], favA[:, :, HD:AW])
                nc.vector.reciprocal(rc[:, 4:8], favB[:, :, HD:AW])
                fsb = npool.tile([P, 8, AW], BF, tag="fsb", name="fsb")
                nc.scalar.copy(fsb[:, 0:4, :], favA[:])
                nc.vector.tensor_copy(fsb[:, 4:8, :], favB[:])
                o_n = npool.tile([P, RQ, P], BF, tag="onorm", name="o_n")
                # Pool's queue also holds the V-path collective lands, which
                # block until the gathers complete; use DVE while those are
                # in flight (pairs 0-2), split DVE/Pool once drained.
                for blk in range(RQ):
                    for h in range(2):
                        reg = (blk % 2) * 2 + h + (0 if blk < 2 else 4)
                        eng = (nc.vector if fin["pair"] < 3 or h == 0
                               else nc.gpsimd)
                        eng.tensor_scalar_mul(
                            o_n[:, blk, h * HD:(h + 1) * HD],
                            fsb[:, reg, 0:HD],
                            rc[:, reg:reg + 1])
                fin["o_n"] = o_n
            elif stage == 1:
                tp = pslot(1, BF, (P, SQ), name="tp")
                for blk in range(RQ):
                    nc.tensor.transpose(tp[:, blk * P:(blk + 1) * P],
                                        fin["o_n"][:, blk, :], id_sb[:])
                fin["tp"] = tp
            else:
                fg, fpi = fin["pair"] // 4, fin["pair"] % 4
                nc.vector.tensor_copy(oT_tiles[fg][:, fpi, :],
                                      fin["tp"][:, 0:SQ])
                fin.clear()

        # qproj injection plan: pairs 0-3 carry two qprojs each (pairs 2-9,
        # second one on slot ppB), pairs 4-9 one each (pairs 10-15).
        qplan = {}
        for p in range(4):
            qplan[p] = (2 * p + 2, 2 * p + 3)
        for p in range(4, 10):
            qplan[p] = (p + 6,)

        for g in range(4):                    # 4 groups of 4 pairs
            oT_tiles[g] = otp.tile([P, RQ, SQ], BF, tag="oT", name=f"oT_{g}")
            for pi in range(4):               # pairs within group
                pair = g * 4 + pi
                wo_cur = [None]
                kc = pair % 4                 # kv chunk holding both kv heads
                kva = 2 * (pair % 4)
                qph = qplan.get(pair, ())

                avA = av.tile([P, 4, AW], FP, tag="av", name="avA")
                avB = av.tile([P, 4, AW], FP, tag="av", name="avB")
                eabs = {}
                for c in range(SC + 2):
                    if c < SC:
                        # per-head 1-bank score tiles: exp(head A) releases
                        # its bank while head B's exp still runs, halving the
                        # psc-WAR granularity that sets the chunk cadence
                        eh = []
                        for hh in range(2):
                            psH = psc.tile([P, SQ], FP, tag="psc", name="psH")
                            nc.tensor.matmul(
                                psH[:],
                                lhsT=kT[hh * 64:hh * 64 + 64, kc,
                                        c * P:(c + 1) * P],
                                rhs=qT[hh * 64:hh * 64 + 64, pair, :],
                                start=True, stop=True,
                                tile_position=(hh * 64, 0))
                            if c in dve_lanes(pair):
                                ebits = epool.tile([P, SQ], I16, tag="exp",
                                                   name="eab")
                                nc.vector.tensor_scalar(ebits[:], psH[:],
                                                        SL, SB, MUL, ADDOP)
                                eh.append(ebits.bitcast(BF))
                            else:
                                eab = epool.tile([P, SQ], BF, tag="exp",
                                                 name="eab")
                                nc.scalar.activation(eab[:], psH[:], EXPF,
                                                     scale=EXP_SCALE)
                                eh.append(eab)
                        eabs[c] = eh
                    # previous pair's normalize/transpose/copy-out: the PE
                    # transposes are IN the in-order PE queue, so they must
                    # not be reached before the cross-engine normalize chain
                    # (recips/copies/muls, ~4.5us) completes - else the whole
                    # PE stalls. pslot(1) is free until c==7 (out-proj) or
                    # c==8 (second qproj).
                    f1c, f2c = 5, 7
                    if c == 0:
                        finish_stage(0)
                    elif c == f1c:
                        finish_stage(1)
                    elif c == f2c:
                        finish_stage(2)
                    if c >= 2:
                        cc_ = c - 2      # attn.V lags two chunks behind exp
                        eh_ = eabs.pop(cc_)
                        # transposed AV: out rows = q positions of one block,
                        # accumulate over kv chunks; a twin N=1 matmul against
                        # the ones column accumulates the softmax denominator
                        # into region col 64.  Region 0 of each bank issues
                        # start (flags the whole bank pending-zero); the rest
                        # ride the flags with start=False.
                        for ti, tl in ((0, avA), (1, avB)):
                            for ri in range(4):
                                blk = ti * 2 + ri // 2
                                h = ri % 2
                                lhsT = eh_[h][:, blk * P:(blk + 1) * P]
                                nc.tensor.matmul(
                                    tl[:, ri, 0:AW], lhsT=lhsT,
                                    rhs=v_pos[:, cc_,
                                              (kva + h) * AW:(kva + h + 1) * AW],
                                    start=(cc_ == 0 and ri == 0),
                                    stop=(cc_ == SC - 1),
                                    skip_group_check=True)
                    # q-projections: 2 dc-chunks per score chunk levels PE's
                    # per-chunk load against the ~1.04us exp cadence
                    if len(qph) >= 1 and 0 <= c <= 7:
                        qproj_mm(qph[0], [c * 2, c * 2 + 1],
                                 pslot(0, name="qps") if c == 0 else None)
                    if len(qph) >= 2 and 8 <= c <= 15:
                        qproj_mm(qph[1], [(c - 8) * 2, (c - 8) * 2 + 1],
                                 pslot(1, name="qps") if c == 8 else None)
                    if c == 3 and g >= 1:
                        wo_cur[0] = wo_load(g - 1, pi)
                    if pair == 15 and c in (3, 5, 9, 11):
                        nf = {3: 0, 5: 1, 9: 2, 11: 3}[c]
                        wo_g3[nf] = wo_load(3, nf)
                    if c == 10 and pair + 1 in qplan:
                        for qp in qplan[pair + 1]:
                            if qp not in wq_tiles and qp not in qproj_ps:
                                wq_prefetch(qp)
                    if c in (7, 9, 11, 13) and g >= 1:
                        out_proj_m(g - 1, pi, wo_cur[0], (c - 7) // 2,
                                   slot=(1 if c in (7, 11) else 0))
                fin.update({"pair": pair, "avA": avA, "avB": avB})

            if g == 3:
                for st in range(3):
                    finish_stage(st, end=True)
                for nf in range(4):
                    for m in range(RQ):
                        out_proj_m(3, nf, wo_g3[nf], m, slot=m % 2)

    nc.compile()
    return nc


_NC = None


def _get_nc():
    global _NC
    if _NC is None:
        _NC = build()
    return _NC


def _host_prep(inputs):
    """Permute wq/wk to half-rotated layout, swizzle all weights into the
    on-chip layouts (so device DMAs are linear), build CC/SS tables, slice
    per-core shards."""
    x = np.asarray(inputs["x"], np.float32)
    cos = np.asarray(inputs["cos"], np.float32)
    sin = np.asarray(inputs["sin"], np.float32)
    wq = np.asarray(inputs["wq"], np.float32)
    wk = np.asarray(inputs["wk"], np.float32)
    wv = np.asarray(inputs["wv"], np.float32)
    wo = np.asarray(inputs["wo"], np.float32)

    def perm_cols(w, nheads):
        idx = np.empty(nheads * HD, np.int64)
        for h in range(nheads):
            idx[h * HD:h * HD + 32] = h * HD + 2 * np.arange(32)
            idx[h * HD + 32:(h + 1) * HD] = h * HD + 2 * np.arange(32) + 1
        return np.ascontiguousarray(w[:, idx])

    wq_p = perm_cols(wq, NQ)
    wk_p = perm_cols(wk, NKV)
    # device layouts
    BFH = ml_dtypes.bfloat16
    wq_dev = np.ascontiguousarray(
        wq_p.reshape(DC, P, DC, P).transpose(2, 1, 0, 3)).astype(BFH)
    wk_dev = np.ascontiguousarray(
        wk_p.reshape(DC, P, KFC, P).transpose(2, 1, 0, 3)).astype(BFH)
    wv_dev = np.ascontiguousarray(
        wv.reshape(DC, P, KF).transpose(1, 0, 2)).astype(BFH)
    wo_dev = np.ascontiguousarray(
        wo.reshape(RQ, RQ, P, RQ, 512).transpose(0, 3, 2, 1, 4)).astype(BFH)

    cosT = np.ascontiguousarray(cos.T)            # [32, S]
    sinT = np.ascontiguousarray(sin.T)
    CC = np.tile(cosT, (4, 1))                    # [128, S]
    # block-pair-swapped so the rope mul reads SS at the SOURCE block's base
    SS = np.concatenate([sinT, -sinT, sinT, -sinT], 0)

    in_maps = []
    for c in range(NCORES):
        b, q = c // GPB, c % GPB
        sl = slice(q * SQ, (q + 1) * SQ)
        x_dev = np.ascontiguousarray(
            x[b, sl, :].T.reshape(DC, P, SQ).transpose(1, 0, 2)).astype(
                ml_dtypes.bfloat16)
        in_maps.append({
            "x": x_dev,
            "wq": wq_dev, "wk": wk_dev, "wv": wv_dev, "wo": wo_dev,
            "cc": np.ascontiguousarray(CC[:, sl]).astype(np.float16),
            "ss": np.ascontiguousarray(SS[:, sl]).astype(np.float16),
        })
    return in_maps


def kernel(**inputs):
    nc = _get_nc()
    in_maps = _host_prep(inputs)
    res = run_bass_kernel_spmd(nc, in_maps, core_ids=list(range(NCORES)))
    out = np.empty((B, S, D), np.float32)
    for c in range(NCORES):
        b, q = c // GPB, c % GPB
        out[b, q * SQ:(q + 1) * SQ, :] = res.results[c]["out"]
    return out

